# revision 34
# baseline (speedup 1.0000x reference)
"""Trainium2 Bass kernel v8 for MatrixMPowerSeriesLayer.

res = sum_{i=0}^{7} C_i @ X^i, batch 64 complex 512x512, data-parallel over
batch on 8 NeuronCores (8 elements/core).  Measured 468.5us (v3 baseline
501.6us), rel_l2 1.324e-2 against the fp32 reference (gate 2e-2).

Transposed Horner:  G_7 = C_7^T;  G_k = C_k^T + X^T @ G_{k+1};  G_0 = res^T.
PE computes X^T @ G with lhsT = X (untransposed), rhs = G.

Changes over v3 (each validated against a perfetto trace):
  - DMA rework: every transfer reads a fully DRAM-contiguous block (strided
    footprints measured ~55-80GB/s vs ~240GB/s contiguous), split across
    both HWDGE rings (SP + Activation).  Each dma_start also costs ~600ns
    serial trigger time on its engine queue plus ~1-2us completion-receipt
    latency, so startup uses a few 256-512KB transfers in exact
    first-consumption order, and b>=1 uses one 768KB/1.5MB transfer per
    tensor group, prefetched a full batch element ahead (xp double-buffered).
    The non-urgent coefficient bulk (cc4..cc0) rides the SP ring between
    b0's fp8 X and the much-later-needed bf16 X.
  - 8 warm-up matmuls on a zeroed scratch tile at kernel start: PE-HAM
    un-throttles (1.2->2.4GHz) after ~3.4us of sustained PE activity, which
    now overlaps the initial DMA wait instead of eating the first ~50 real
    matmuls.
  - First chunk of each fp8 step reorders its 8 MMs so the two MMs whose
    rhs is the previous step's last DVE output run last (~1.3us of cover
    vs ~650ns before): removes a ~180ns PE stall per step boundary.

Rejected experiments (see memory notes): 6 fp8 steps = 478us but rel_l2
2.017e-2 (X-quant and G-storage-quant each ~1.4e-2, both irreducible at
e4m3); e3m4 would fit the error budget but DoubleRow is fp8e4/e5-only;
int8 matmul unsupported by bass; 3-product (Karatsuba/Knuth) fp8 complex
multiply is DVE-bound (fp8-out DVE ops run 1x mode); bf16 PSUM matmul
accumulation is TRN3-only.

Precision scheme (unchanged):
  - fp8 steps: e4m3 DoubleRow schoolbook complex multiply; DR runs
    contraction 256 per MM at the same 216ns cadence as a regular MM.
    Schoolbook (4 products, with -Xi precomputed) needs no Gr+Gi running
    sum and only one DVE op per produced component:
      T_i = Xr^T Gi + Xi^T Gr      (4 DR MMs, one PSUM bank)
      T_r = Xr^T Gr + (-Xi)^T Gi   (4 DR MMs)
      Gi' = Ci^T + T_i ; Gr' = Cr^T + T_r   (stt, scale folded)
    X shipped as fp8(64*X), G stored as fp8(16*G); products in PSUM carry
    2^10; the stt unscale keeps the next G at 16x (1x at the transition).
  - step k=0: bf16 Karatsuba (3 products = 12 MMs/chunk), PSUM banks
    (T3, T1, T2), ScalarE stages (w,u,v), DVE combines.
"""

import numpy as np
import ml_dtypes
from contextlib import ExitStack

import concourse.bass as bass
from concourse import bacc
import concourse.mybir as mybir
import concourse.tile as tile
from concourse.bass_utils import run_bass_kernel_spmd

B, N, DEG = 64, 512, 8
P = 128
KO = N // P
NCORES = 8
BPC = B // NCORES
F32 = mybir.dt.float32
BF16 = mybir.dt.bfloat16
FP8 = mybir.dt.float8e4
BF16_NP = ml_dtypes.bfloat16
FP8_NP = ml_dtypes.float8_e4m3

N_FP8_STEPS = 5          # steps k = DEG-2 .. DEG-1-N_FP8_STEPS run in fp8 DR
                         # (6 measured 2.017e-2 on the 2e-2 gate -- rejected;
                         #  e3m4 would fit the budget but DR is e4m3/e5m2-only)
N_WARM_MM = 8            # warm-up matmuls to release the PE HAM throttle
DR = mybir.MatmulPerfMode.DoubleRow

_NC_CACHE: dict = {}


def _build_nc(bpc: int = BPC, deg: int = DEG) -> bass.Bass:
    nc = bacc.Bacc()
    fp8_min_k = deg - 1 - N_FP8_STEPS    # k >= fp8_min_k -> fp8 step

    # X packed [b, t, ko, p, n], t = (Xr, Xi, -Xi)*64 fp8 / (Xr, Xi, Xr+Xi)
    # bf16.  Every dma_start below reads a fully DRAM-contiguous block --
    # strided DRAM footprints measured ~55-80GB/s vs ~240GB/s contiguous.
    # b=0 is also shipped as its own tensor so it can stream at 64KB
    # [t, ko] granularity in exact first-consumption order (subtile deps
    # release the first matmuls as each chunk lands); b>=1 load as coarse
    # 256KB/1.5MB transfers prefetched a full batch element ahead.
    # NOTE on DMA access patterns: the DRAM-side AP must iterate with the
    # partition dim outermost (matching the SBUF side) -- balance_dma_aps
    # does not permute dims.  So coarse transfers use partition-major DRAM
    # layouts sliced only at the outermost (b) dim, keeping the footprint
    # fully contiguous; fine 64KB transfers use [.., P, N] layouts.
    # A dma_start trigger also costs ~600ns SERIAL time on the issuing
    # engine's queue, so startup wants a few medium transfers, not many
    # small ones: b0's X as 3x256KB (per tensor), g0c8/cc6 as one transfer
    # each.  b=0 ships in its own per-tensor-sliceable layout.
    x80_d = nc.declare_dram_parameter("x80", [3, P, KO, N], FP8, isOutput=False)
    x8_d = nc.declare_dram_parameter("x8", [bpc, P, 3, KO, N], FP8, isOutput=False)
    xb_d = nc.declare_dram_parameter("xb", [bpc, P, 3, KO, N], BF16, isOutput=False)
    # coeffs for steps k < deg-2, coarse partition-major: [k, p, ko, j, n]
    ccr_d = nc.declare_dram_parameter("ccr", [deg - 2, P, KO, 2, N], BF16, isOutput=False)
    # first-step (k = deg-2) coeffs: [p, ko, j, n]
    cc6_d = nc.declare_dram_parameter("cc6", [P, KO, 2, N], BF16, isOutput=False)
    # G init (C7^T), fp8(16x), packed (Gi, Gr)
    g0c8_d = nc.declare_dram_parameter("g0c8", [P, KO, 2, N], FP8, isOutput=False)

    # output: packed (imag, real) bf16, host up-casts + transposes
    oc_d = nc.declare_dram_parameter("oc", [bpc, KO, P, 2, N], BF16, isOutput=True)

    with tile.TileContext(nc) as tc, ExitStack() as ctx:
        cp = ctx.enter_context(tc.tile_pool(name="cp", bufs=1))
        xp = ctx.enter_context(tc.tile_pool(name="xp", bufs=2))
        gp = ctx.enter_context(tc.tile_pool(name="gp", bufs=2))
        up = ctx.enter_context(tc.tile_pool(name="up", bufs=6))
        ps = ctx.enter_context(tc.tile_pool(name="ps", bufs=2, space="PSUM"))

        cc = cp.tile([P, deg - 1, KO, 2, N], BF16, name="cc")
        g0c8 = cp.tile([P, KO, 2, N], FP8, name="g0c8")
        wsc = cp.tile([P, 5 * P], BF16, name="wsc")

        # --- PE warm-up: no data deps beyond the memset, so these schedule
        # ahead of every real matmul and overlap the initial DMA wait.
        nc.vector.memset(wsc[:], 0.001)
        wt = ps.tile([P, 2, N], F32, tag="t", bufs=3, name="warm")
        for _ in range(N_WARM_MM):
            nc.tensor.matmul(wt[:, 0, :], lhsT=wsc[:, :P], rhs=wsc[:, P:],
                             start=True, stop=True)

        # --- DMA emission in need-order ---
        # Activation HWDGE ring: G7 + first-step coefficients at 64KB
        # granularity in first-consumption order, then the rest of cc
        # coarse.  No tile WAR waits, so these post immediately and stream
        # in parallel with the SP ring.
        # Startup-critical bytes only: G7 + the first two steps' coeffs.
        nc.scalar.dma_start(out=g0c8[:], in_=g0c8_d[:, :, :, :])
        nc.scalar.dma_start(out=cc[:, deg - 2], in_=cc6_d[:, :, :, :])
        nc.scalar.dma_start(out=cc[:, deg - 3], in_=ccr_d[deg - 3])

        # SP HWDGE ring: X tiles (+ outputs, emitted during compute).
        def load_x8(b):
            t8 = xp.tile([P, 3, KO, N], FP8, tag="x8", name=f"x8_{b}")
            if b == 0:
                # per-tensor transfers in first-consumption order
                # (finer splits inflate per-MM subtile-dep sem-waits)
                for t in (0, 1, 2):
                    nc.sync.dma_start(out=t8[:, t], in_=x80_d[t])
            else:
                nc.sync.dma_start(out=t8[:], in_=x8_d[b])
            return t8

        def load_xb(b):
            tb = xp.tile([P, 3, KO, N], BF16, tag="xb", name=f"xb_{b}")
            nc.sync.dma_start(out=tb[:], in_=xb_d[b])
            return tb

        x80t = load_x8(0)
        # Remaining coeffs ride the SP ring BETWEEN b0's fp8 X (which must
        # land by ~13us) and the much-later-needed bf16 X -- keeping the
        # startup window uncontended while still arriving a step ahead.
        for k in range(deg - 4, -1, -1):
            nc.sync.dma_start(out=cc[:, k], in_=ccr_d[k])
        x_tiles = {0: (x80t, load_xb(0))}

        for b in range(bpc):
            cur8, curb = x_tiles.pop(b)
            if b + 1 < bpc:
                x_tiles[b + 1] = (load_x8(b + 1), load_xb(b + 1))
            xr8, xi8, xn8 = cur8[:, 0], cur8[:, 1], cur8[:, 2]
            xr, xi, xs = curb[:, 0], curb[:, 1], curb[:, 2]

            g2 = None          # [P, KO, 2, N] (Gi, Gr), fp8 or bf16
            gs = None          # [P, KO, N] bf16 (Karatsuba steps only)
            for k in range(deg - 2, -1, -1):
                fp8_step = k >= fp8_min_k
                trans = k == fp8_min_k       # last fp8 step: emit bf16 + gs
                last = k == 0

                if fp8_step:
                    out_dt = BF16 if trans else FP8
                    out_tag = "g2b" if trans else "g28"
                    g2_n = gp.tile([P, KO, 2, N], out_dt, tag=out_tag,
                                   name=f"g2_{b}_{k}")
                    gs_n = (
                        gp.tile([P, KO, N], BF16, tag="gs", name=f"gs{b}_{k}")
                        if trans else None
                    )
                    for m in range(KO):
                        msl = slice(m * P, (m + 1) * P)
                        # one 2-bank tile per chunk, triple-buffered:
                        # reuse distance 3 chunks >> the stt/Act drain time.
                        t = ps.tile([P, 2, N], F32, tag="t", bufs=3, name=f"t_{b}_{k}_{m}")
                        # T_i -> slice 0, T_r -> slice 1.
                        # Four DR accumulation pairs per slice; (slice, lhsT,
                        # j-of-rhs): Tr = xr8*G[j1] + xn8*G[j0],
                        # Ti = xr8*G[j0] + xi8*G[j1].
                        gq = g0c8 if g2 is None else g2
                        mms = {
                            "A": (1, xr8, 1), "B": (1, xn8, 0),
                            "C": (0, xr8, 0), "D": (0, xi8, 1),
                        }
                        if b == 0 and k == deg - 2 and m == 0:
                            # Very first chunk: rhs is g0c8 (always ready);
                            # the gate is the three X transfers landing in
                            # order xr8, xi8, xn8.  Run all xr8 MMs first.
                            order = [("A", 0), ("C", 0), ("A", 1), ("C", 1),
                                     ("D", 0), ("D", 1), ("B", 0), ("B", 1)]
                        elif m == 0:
                            # First chunk of a step: the c=1 (ko 2:4) rhs
                            # slices against G[j0] are the previous step's
                            # last DVE outputs (ready ~658ns after its last
                            # MM).  Emit all c=0 work plus the c=1 j1 MMs
                            # first (~1.3us of cover), the two c=1 j0 MMs
                            # last.  T_r still finishes early enough for its
                            # ScalarE staging to overlap.
                            order = [("A", 0), ("C", 0), ("D", 0), ("B", 0),
                                     ("A", 1), ("D", 1), ("B", 1), ("C", 1)]
                        else:
                            order = [("A", 0), ("A", 1), ("B", 0), ("B", 1),
                                     ("C", 0), ("C", 1), ("D", 0), ("D", 1)]
                        started = set()
                        seen = {q: 0 for q in mms}
                        for q, c in order:
                            s, xt_, j = mms[q]
                            pr = slice(2 * c, 2 * c + 2)
                            first = s not in started
                            started.add(s)
                            seen[q] += 1
                            # stop on the last-emitted MM of each slice's
                            # accumulation group (B ends Tr, C or D ends Ti)
                            is_last = (
                                seen["B"] == 2 and s == 1 and q == "B"
                                if s == 1 else
                                seen["C"] + seen["D"] == 4 and s == 0
                            )
                            nc.tensor.matmul(
                                t[:, s, :], lhsT=xt_[:, pr, msl],
                                rhs=gq[:, pr, j, :],
                                start=first, stop=is_last, perf_mode=DR,
                            )

                        # G' = (T_psum * unscale) + C directly on the DVE.
                        # X is shipped as fp8(64*X), G stored as fp8(16*G), so
                        # a product in PSUM carries 2^10; unscale keeps the
                        # next G at 16x (fp8 steps) or 1x (transition).
                        unscale = 2.0 ** -10 if trans else 2.0 ** -6
                        if m < KO - 1:
                            # ScalarE (idle in fp8 steps) stages T_r with the
                            # free affine scale; DVE then does a cheap bf16
                            # 2x add.  T_i stays a PSUM-direct stt on DVE.
                            ur = up.tile([P, N], BF16, tag="ur8",
                                         name=f"ur8_{b}_{k}_{m}")
                            nc.scalar.activation(
                                ur[:], t[:, 1, :],
                                mybir.ActivationFunctionType.Copy,
                                scale=unscale,
                            )
                            nc.vector.tensor_add(
                                g2_n[:, m, 1, :], ur[:], cc[:, k, m, 1, :]
                            )
                        else:
                            # last chunk: PSUM-direct stt gives the shortest
                            # tail into the next step's MMs
                            nc.vector.scalar_tensor_tensor(
                                g2_n[:, m, 1, :], t[:, 1, :], unscale,
                                cc[:, k, m, 1, :],
                                op0=mybir.AluOpType.mult, op1=mybir.AluOpType.add,
                            )
                        nc.vector.scalar_tensor_tensor(
                            g2_n[:, m, 0, :], t[:, 0, :], unscale,
                            cc[:, k, m, 0, :],
                            op0=mybir.AluOpType.mult, op1=mybir.AluOpType.add,
                        )
                        if trans:
                            nc.vector.tensor_add(
                                gs_n[:, m, :], g2_n[:, m, 0, :], g2_n[:, m, 1, :]
                            )
                    g2, gs = g2_n, gs_n

                else:
                    # bf16 Karatsuba step; PSUM order (T3, T1, T2)
                    if not last:
                        g2_n = gp.tile([P, KO, 2, N], BF16, tag="g2b",
                                       name=f"g2_{b}_{k}")
                        gs_n = gp.tile([P, KO, N], BF16, tag="gs",
                                       name=f"gs{b}_{k}")
                    for m in range(KO):
                        msl = slice(m * P, (m + 1) * P)
                        ta = ps.tile([P, 2, N], F32, tag="t", bufs=3, name=f"ta_{b}_{k}_{m}")
                        tb = ps.tile([P, N], F32, tag="t3", bufs=2, name=f"tb_{b}_{k}_{m}")
                        dst = {1: ta[:, 0, :], 2: ta[:, 1, :], 0: tb[:]}
                        # products: T1 = Xr Gr -> slice 1, T2 = Xi Gi -> 2,
                        # T3 = Xs Gs -> slice 0.  For the first chunk of a
                        # step, push every product's ko=3 MM to the end: the
                        # ko=3 G-slices are the previous step's last DVE
                        # outputs and arrive latest.
                        prods = [
                            (1, xr, lambda ko: g2[:, ko, 1, :]),
                            (2, xi, lambda ko: g2[:, ko, 0, :]),
                            (0, xs, lambda ko: gs[:, ko, :]),
                        ]
                        if m == 0:
                            order = [(s, ko) for s, _, _ in prods for ko in range(KO - 1)]
                            order += [(s, KO - 1) for s, _, _ in prods]
                        else:
                            order = [(s, ko) for s, _, _ in prods for ko in range(KO)]
                        pmap = {s: (xt_, rhs_) for s, xt_, rhs_ in prods}
                        for s, ko in order:
                            xt_, rhs_ = pmap[s]
                            nc.tensor.matmul(
                                dst[s], lhsT=xt_[:, ko, msl],
                                rhs=rhs_(ko),
                                start=(ko == 0), stop=(ko == KO - 1),
                            )

                        uvw = up.tile([P, 3, N], BF16, tag="uvw",
                                      name=f"uvw_{b}_{k}_{m}")
                        nc.scalar.copy(uvw[:, 1:3, :], ta[:])
                        nc.scalar.copy(uvw[:, 0, :], tb[:])
                        w, u, v = uvw[:, 0, :], uvw[:, 1, :], uvw[:, 2, :]

                        # e2 = (w-u, u-v) = (c0, a); then c0 -= v -> c1
                        e2 = up.tile([P, 2, N], BF16, tag="e2",
                                     name=f"e2_{b}_{k}_{m}")
                        nc.vector.tensor_sub(e2[:], uvw[:, 0:2, :], uvw[:, 1:3, :])
                        nc.vector.tensor_sub(e2[:, 0, :], e2[:, 0, :], v)
                        if last:
                            f2 = up.tile([P, 2, N], BF16, tag="f2",
                                         name=f"f2_{b}_{m}")
                            nc.vector.tensor_add(f2[:], e2[:], cc[:, k, m, :, :])
                            nc.sync.dma_start(out=oc_d[b, m], in_=f2[:])
                        else:
                            nc.vector.tensor_add(
                                g2_n[:, m, :, :], e2[:], cc[:, k, m, :, :]
                            )
                            nc.vector.tensor_add(
                                gs_n[:, m, :], g2_n[:, m, 0, :], g2_n[:, m, 1, :]
                            )
                    if not last:
                        g2, gs = g2_n, gs_n

    nc.finalize()
    return nc


def _get_nc() -> bass.Bass:
    if "nc" not in _NC_CACHE:
        _NC_CACHE["nc"] = _build_nc()
    return _NC_CACHE["nc"]


def _prep_inputs(x: np.ndarray, coeffs: np.ndarray):
    x = np.ascontiguousarray(x, dtype=np.float32)
    coeffs = np.ascontiguousarray(coeffs, dtype=np.float32)

    xr_f = x[:, 0].reshape(B, KO, P, N)
    xi_f = x[:, 1].reshape(B, KO, P, N)
    # [B, P, 3, KO, N] partition-major for the coarse per-b transfers
    xr_p = xr_f.transpose(0, 2, 1, 3)
    xi_p = xi_f.transpose(0, 2, 1, 3)

    xb = np.empty((B, P, 3, KO, N), dtype=BF16_NP)
    xb[:, :, 0] = xr_p.astype(BF16_NP)
    xb[:, :, 1] = xi_p.astype(BF16_NP)
    xb[:, :, 2] = (xr_p + xi_p).astype(BF16_NP)
    # fp8 operands are pre-scaled into e4m3's normal range (X entries are
    # ~N(0, 0.02^2) -- raw they'd be almost entirely subnormal):
    #   X shipped as fp8(64*X), G kept as fp8(16*G).
    x8 = np.empty((B, P, 3, KO, N), dtype=FP8_NP)
    x8[:, :, 0] = (xr_p * 64).astype(FP8_NP)
    x8[:, :, 1] = (xi_p * 64).astype(FP8_NP)
    x8[:, :, 2] = (-xi_p * 64).astype(FP8_NP)
    # b=0 per core, per-tensor-sliceable [3, P, KO, N]
    x80 = np.empty((NCORES, 3, P, KO, N), dtype=FP8_NP)
    x80[:, 0] = (xr_p[:: BPC] * 64).astype(FP8_NP)
    x80[:, 1] = (xi_p[:: BPC] * 64).astype(FP8_NP)
    x80[:, 2] = (-xi_p[:: BPC] * 64).astype(FP8_NP)

    crT = np.ascontiguousarray(coeffs[:, 0].transpose(0, 2, 1))  # [DEG, N, N]
    ciT = np.ascontiguousarray(coeffs[:, 1].transpose(0, 2, 1))
    # cc[k, ko, j, p, n]: j=0 -> Ci^T, j=1 -> Cr^T.  For the non-transition
    # fp8 steps (k > k_trans) the coefficients carry the 16x G-scale.
    k_trans = DEG - 1 - N_FP8_STEPS
    cc_f = np.stack(
        [ciT[: DEG - 1].reshape(DEG - 1, KO, P, N),
         crT[: DEG - 1].reshape(DEG - 1, KO, P, N)], axis=2
    ).copy()                                              # [DEG-1, KO, 2, P, N]
    cc_f[k_trans + 1:] *= 16.0
    cc6 = np.ascontiguousarray(
        cc_f[DEG - 2].transpose(2, 0, 1, 3).astype(BF16_NP)   # [P, KO, 2, N]
    )
    ccr = np.ascontiguousarray(
        cc_f[: DEG - 2].transpose(0, 3, 1, 2, 4).astype(BF16_NP)
    )                                                     # [DEG-2, P, KO, 2, N]
    g0c8 = np.ascontiguousarray(
        (16.0 * np.stack(
            [ciT[DEG - 1].reshape(KO, P, N), crT[DEG - 1].reshape(KO, P, N)],
            axis=1,
        ).transpose(2, 0, 1, 3)).astype(FP8_NP)           # [P, KO, 2, N]
    )

    in_maps = []
    for c in range(NCORES):
        sl = slice(c * BPC, (c + 1) * BPC)
        in_maps.append(
            {
                "x80": np.ascontiguousarray(x80[c]),
                "x8": np.ascontiguousarray(x8[sl]),
                "xb": np.ascontiguousarray(xb[sl]),
                "ccr": ccr,
                "cc6": cc6,
                "g0c8": g0c8,
            }
        )
    return in_maps


def _assemble_output(results) -> np.ndarray:
    out = np.empty((B, 2, N, N), dtype=np.float32)
    for c in range(NCORES):
        oc = results[c]["oc"].astype(np.float32)      # [BPC, KO, P, 2, N]
        re = oc[:, :, :, 1, :].reshape(BPC, N, N)     # res^T rows = ko*P+p
        im = oc[:, :, :, 0, :].reshape(BPC, N, N)
        for b in range(BPC):
            out[c * BPC + b, 0] = re[b].T
            out[c * BPC + b, 1] = im[b].T
    return out


def run_sharded(x: np.ndarray, coeffs: np.ndarray, **run_kwargs):
    nc = _get_nc()
    in_maps = _prep_inputs(x, coeffs)
    res = run_bass_kernel_spmd(nc, in_maps, list(range(NCORES)), **run_kwargs)
    return _assemble_output(res.results), res


def kernel(x: np.ndarray, coeffs: np.ndarray) -> np.ndarray:
    out, _ = run_sharded(x, coeffs)
    return out
